# revision 28
# baseline (speedup 1.0000x reference)
"""Trainium2 Bass kernel for KBLAM Gemma3n attention (B=2, S=1024, H=2048,
NH=16, NKV=4, HD=128, KB=1024), sharded over 8 NeuronCores as
(batch x kv-head-group): core = 4*b + g handles batch b and kv head g
(which serves q-heads 4g..4g+3).  Each core computes a partial s-major
output y_part (S, H) = attn_out @ Wo[:, 512g:512g+512].T ; the host sums
the 4 partials per batch.

v6 design notes:
 - projections run bf16 (xT + W host-cast): halves the startup DMA;
   k and v chunks interleave per h-tile to ride the xT arrival; the
   first transfers split across both HWDGE queues (sync + scalar).
 - scores / rope / normalization / output projection stay f32r (same
   PE rate as bf16 at N=512, better numerics).
 - exp output (at) and values (vkm/kbv) are bf16: attn@v keeps full PE
   rate and the DVE 2x mode makes the self-denominator adds cheap.
 - softmax denominator is HYBRID: the 8 KB steps accumulate on the PE
   ([1,512] ones-matmuls into one PSUM group, no DVE chain), the 4-8
   self steps accumulate on the DVE in bf16, folded back into the same
   PSUM group with one final ones-matmul.  This balances PE vs DVE vs
   ACT, each of which otherwise becomes the attention bottleneck.
 - exp activations are paired ([128,1024] PSUM tiles, one ACTIVATE for
   two score steps, trimmed to the written region) to amortize the
   352-cycle ACT instruction overhead.
"""
import math
from contextlib import ExitStack

import numpy as np

B, S, H = 2, 1024, 2048
NH, NKV, HD = 16, 4, 128
KB = 1024
THETA = 10000.0
SCALE = 1.0 / math.sqrt(HD)


def _build_program(self_tiles, mixed_idx, n_mask, col0_map):
    """Build the single-core Bass/Tile program."""
    import concourse.tile as tile
    from concourse import bacc, mybir

    f32 = mybir.dt.float32
    f32r = mybir.dt.float32r
    bf16 = mybir.dt.bfloat16
    nc = bacc.Bacc("TRN2", target_bir_lowering=False, debug=False,
                   enable_asserts=False, num_devices=8)

    xT = nc.dram_tensor("xT", [H, S], bf16, kind="ExternalInput")
    # packed weights: per-dt blocks of 16 h-tiles: cols 2048*dt + 128*h
    wq = nc.dram_tensor("wq", [128, 8192], bf16, kind="ExternalInput")
    wqn = nc.dram_tensor("wqn", [128, 8192], bf16, kind="ExternalInput")
    wk = nc.dram_tensor("wk", [128, 2048], bf16, kind="ExternalInput")
    wv = nc.dram_tensor("wv", [128, 2048], bf16, kind="ExternalInput")
    # wo packed: block i at cols 2048*i = Wo_g^T[128i:128i+128, :]
    wo = nc.dram_tensor("wo", [128, 8192], f32r, kind="ExternalInput")
    kbkT = nc.dram_tensor("kbkT", [128, KB], bf16, kind="ExternalInput")
    # kbv packed key-major tiles side by side: tile t at cols 128*t
    kbv = nc.dram_tensor("kbv", [128, KB], bf16, kind="ExternalInput")
    cosT = nc.dram_tensor("cosT", [128, S], f32r, kind="ExternalInput")
    sinT = nc.dram_tensor("sinT", [128, S], f32r, kind="ExternalInput")
    ropePT = nc.dram_tensor("ropePT", [128, 128], f32r, kind="ExternalInput")
    onesb = nc.dram_tensor("onesb", [128, 128], bf16, kind="ExternalInput")
    onesf = nc.dram_tensor("onesf", [128, 128], f32r, kind="ExternalInput")
    identb = nc.dram_tensor("identb", [128, 128], bf16, kind="ExternalInput")
    if n_mask:
        masks = nc.dram_tensor("masks", [128, 512 * n_mask], bf16,
                               kind="ExternalInput")
    y = nc.dram_tensor("y", [S, H], f32, kind="ExternalOutput")

    with tile.TileContext(nc) as tc, ExitStack() as ctx:
        po = ctx.enter_context(tc.tile_pool(name="projout", bufs=1))
        qTr = po.tile([128, 4096], bf16, tag="qTr")
        qnT = po.tile([128, 4096], bf16, tag="qnT")
        kTr = po.tile([128, 1024], bf16, tag="kTr")
        vkm = po.tile([128, 1024], bf16, tag="vkm")

        consts = ctx.enter_context(tc.tile_pool(name="consts", bufs=1))
        kbp = ctx.enter_context(tc.tile_pool(name="kb", bufs=1))

        # ---------------- phase 1: projections + rope + v transpose ------
        with tc.tile_pool(name="xw", bufs=1) as xw, \
             tc.tile_pool(name="wt", bufs=3) as wpool, \
             tc.tile_pool(name="ptmp", bufs=3) as ptmp, \
             tc.tile_pool(name="psr", bufs=2, space="PSUM") as psr:
            # k-chunk deps first, split across BOTH HWDGE queues so the
            # first matmul fires early; rope constants (needed ~20us+)
            # are pushed after the xt tiles.
            wblk_k = wpool.tile([128, 2048], bf16, tag="wblk", name="wblk_k")
            nc.sync.dma_start(wblk_k[:, 0:512], wk[:, 0:512])
            xt = xw.tile([128, 16384], bf16, tag="xt")
            nc.scalar.dma_start(xt[:, 0:512], xT[0:128, 0:512])
            nc.sync.dma_start(xt[:, 512:1024], xT[0:128, 512:1024])
            nc.scalar.dma_start(wblk_k[:, 512:1024], wk[:, 512:1024])
            nc.sync.dma_start(wblk_k[:, 1024:1536], wk[:, 1024:1536])
            nc.scalar.dma_start(wblk_k[:, 1536:2048], wk[:, 1536:2048])
            wblk_v = wpool.tile([128, 2048], bf16, tag="wblk", name="wblk_v")
            nc.sync.dma_start(wblk_v[:, 0:1024], wv[:, 0:1024])
            nc.scalar.dma_start(wblk_v[:, 1024:2048], wv[:, 1024:2048])
            wblk_q0 = wpool.tile([128, 2048], bf16, tag="wblk",
                                 name="wblk_q0")
            for h in range(1, 16):
                eng = nc.sync if h % 2 == 0 else nc.scalar
                eng.dma_start(xt[:, 1024 * h:1024 * h + 1024],
                              xT[128 * h:128 * h + 128, :])
                if h == 7:
                    # first q/qn weight blocks early: the q0 chunk starts
                    # right after k+v and must not wait behind consts
                    nc.sync.dma_start(wblk_q0[:], wq[:, 0:2048])
            rp_sb = consts.tile([128, 128], f32r, tag="rp")
            nc.sync.dma_start(rp_sb[:], ropePT[:])
            id_sb = consts.tile([128, 128], bf16, tag="id")
            nc.scalar.dma_start(id_sb[:], identb[:])
            onb_sb = consts.tile([128, 128], bf16, tag="onesb")
            nc.sync.dma_start(onb_sb[:], onesb[:])
            onf_sb = consts.tile([128, 128], f32r, tag="onesf")
            nc.scalar.dma_start(onf_sb[:], onesf[:])
            cos_sb = consts.tile([128, S], f32r, tag="cos")
            nc.sync.dma_start(cos_sb[:], cosT[:])
            sin_sb = consts.tile([128, S], f32r, tag="sin")
            nc.scalar.dma_start(sin_sb[:], sinT[:])
            vt_tmp = xw.tile([128, 1024], bf16, tag="vt")

            def rope_chunk(ps, half, dst):
                tmp = ptmp.tile([128, 512], f32r, tag="tmp")
                nc.scalar.copy(tmp[:], ps[:])
                pp = psr.tile([128, 512], f32, tag="pp")
                nc.tensor.matmul(pp[:], rp_sb[:], tmp[:], start=True, stop=True)
                cs = cos_sb[:, 512 * half:512 * half + 512]
                sn = sin_sb[:, 512 * half:512 * half + 512]
                t3 = ptmp.tile([128, 512], f32r, tag="t3")
                nc.vector.tensor_mul(t3[:], tmp[:], cs)
                tmp2 = ptmp.tile([128, 512], f32r, tag="tmp2")
                nc.vector.tensor_mul(tmp2[:], pp[:], sn)
                nc.vector.tensor_add(dst, t3[:], tmp2[:])

            # ---- k and v interleaved per h-tile: rides the xT DMA ----
            with tc.tile_pool(name="pskv", bufs=1, space="PSUM") as pskv:
                pss_k = [pskv.tile([128, 512], f32, tag="pk0", name="pk0"),
                         pskv.tile([128, 512], f32, tag="pk1", name="pk1")]
                pss_v = [pskv.tile([128, 512], f32, tag="pv0", name="pv0"),
                         pskv.tile([128, 512], f32, tag="pv1", name="pv1")]
                for h in range(16):
                    for half in range(2):
                        nc.tensor.matmul(
                            pss_k[half][:], wblk_k[:, 128 * h:128 * h + 128],
                            xt[:, 1024 * h + 512 * half:
                               1024 * h + 512 * half + 512],
                            start=(h == 0), stop=(h == 15))
                    for half in range(2):
                        nc.tensor.matmul(
                            pss_v[half][:], wblk_v[:, 128 * h:128 * h + 128],
                            xt[:, 1024 * h + 512 * half:
                               1024 * h + 512 * half + 512],
                            start=(h == 0), stop=(h == 15))
                for half in range(2):
                    rope_chunk(pss_k[half], half,
                               kTr[:, 512 * half:512 * half + 512])
                for half in range(2):
                    nc.scalar.copy(vt_tmp[:, 512 * half:512 * half + 512],
                                   pss_v[half][:])
                for t in range(8):
                    pst = psr.tile([128, 128], bf16, tag="ptr")
                    nc.tensor.transpose(
                        pst[:], vt_tmp[:, 128 * t:128 * t + 128], id_sb[:])
                    nc.scalar.copy(vkm[:, 128 * t:128 * t + 128], pst[:])

            kbk_sb = kbp.tile([128, KB], bf16, tag="kbk")
            kbv_sb = kbp.tile([128, KB], bf16, tag="kbv")
            if n_mask:
                mask_sb = consts.tile([128, 512 * n_mask], bf16, tag="mask")

            # ---- q / qn chunks, sequential (xT fully resident by now) ----
            with tc.tile_pool(name="psq", bufs=4, space="PSUM") as psq:
                chunks = []
                for i in range(4):
                    chunks.append((wq, i, 'q'))
                    chunks.append((wqn, i, 'qn'))
                for ci, (w_dram, dt_i, kind) in enumerate(chunks):
                    if ci == 0:
                        wblk = wblk_q0  # preloaded during the xT stream
                    else:
                        wblk = wpool.tile([128, 2048], bf16, tag="wblk",
                                          name="wblk")
                        nc.sync.dma_start(
                            wblk[:],
                            w_dram[:, 2048 * dt_i:2048 * dt_i + 2048])
                    # attention-phase loads interleave on the scalar queue
                    # AFTER the q weights so they never starve projections
                    if ci == 1:
                        nc.scalar.dma_start(kbk_sb[:], kbkT[:])
                        nc.scalar.dma_start(kbv_sb[:], kbv[:])
                    elif ci == 2 and n_mask:
                        nc.scalar.dma_start(mask_sb[:], masks[:])
                    pss = [psq.tile([128, 512], f32, tag="pq", name="pq0"),
                           psq.tile([128, 512], f32, tag="pq", name="pq1")]
                    for h in range(16):
                        for half in range(2):
                            nc.tensor.matmul(
                                pss[half][:], wblk[:, 128 * h:128 * h + 128],
                                xt[:, 1024 * h + 512 * half:
                                   1024 * h + 512 * half + 512],
                                start=(h == 0), stop=(h == 15))
                    for half in range(2):
                        if kind == 'q':
                            dst = qTr[:, 1024 * dt_i + 512 * half:
                                      1024 * dt_i + 512 * half + 512]
                            rope_chunk(pss[half], half, dst)
                        else:
                            nc.scalar.copy(
                                qnT[:, 1024 * dt_i + 512 * half:
                                    1024 * dt_i + 512 * half + 512],
                                pss[half][:])

        # ---------------- phase 2: attention ------------------------------
        onp = ctx.enter_context(tc.tile_pool(name="onp", bufs=1))
        outn = onp.tile([128, 4096], f32r, tag="outn")
        wo_sb = onp.tile([128, 8192], f32r, tag="wo")
        # split across both queues; needed only when y emission starts
        nc.sync.dma_start(wo_sb[:, 0:4096], wo[:, 0:4096])
        nc.scalar.dma_start(wo_sb[:, 4096:8192], wo[:, 4096:8192])

        with tc.tile_pool(name="at", bufs=18) as atp, \
             tc.tile_pool(name="nrm", bufs=2) as nrm, \
             tc.tile_pool(name="psaux", bufs=1, space="PSUM") as psaux, \
             tc.tile_pool(name="psout", bufs=1, space="PSUM") as psout, \
             tc.tile_pool(name="ysb", bufs=2) as ysbp:

            def emit_y_tile(st, psy, tail):
                cy, off = st // 4, 128 * (st % 4)
                ysb = ysbp.tile([128, 2048], f32, tag="ysb", name="ysb")
                for n in range(4):
                    py = psy.tile([128, 512], f32, tag="y", name="py")
                    for i in range(4):
                        lcol = 1024 * i + 512 * cy + off
                        nc.tensor.matmul(
                            py[:], outn[:, lcol:lcol + 128],
                            wo_sb[:, 2048 * i + 512 * n:
                                  2048 * i + 512 * n + 512],
                            start=(i == 0), stop=(i == 3))
                    if tail and n % 2 == 1:
                        nc.scalar.copy(ysb[:, 512 * n:512 * n + 512],
                                       py[:])
                    else:
                        nc.vector.tensor_copy(
                            ysb[:, 512 * n:512 * n + 512], py[:])
                    deng = nc.scalar if (tail and n % 2 == 0) else nc.sync
                    deng.dma_start(
                        y[128 * st:128 * st + 128,
                          512 * n:512 * n + 512],
                        ysb[:, 512 * n:512 * n + 512])

            # chunk-finish state carried into the NEXT chunk so the
            # fold/reciprocal/broadcast chain (and its DVE dependencies)
            # never head-of-line-blocks the in-order PE queue at a chunk
            # boundary.
            pending = []

            def flush_finish(psy):
                if not pending:
                    return
                st = pending.pop()
                for (ats, off, w, c0, first) in st['deferred']:
                    if first:
                        nc.vector.tensor_copy(st['acc'][:],
                                              ats[:, off:off + 512])
                    else:
                        nc.vector.tensor_add(
                            st['acc'][:, c0:512], st['acc'][:, c0:512],
                            ats[:, off:off + w])
                # fold the self accumulator into the PSUM den group
                nc.tensor.matmul(st['aux'][0:1, 0:512], onb_sb[:, 0:1],
                                 st['acc'][:], start=False, stop=True)
                rec32 = nrm.tile([1, 512], f32, tag="rec32")
                nc.vector.reciprocal_approx_fast(rec32[:],
                                                 st['aux'][0:1, :])
                rec = nrm.tile([1, 512], f32r, tag="rec")
                nc.vector.tensor_copy(rec[:], rec32[:])
                nc.tensor.matmul(st['aux'][:, 0:512], onf_sb[0:1, :],
                                 rec[:], start=True, stop=True)
                bc_sb = nrm.tile([128, 512], f32r, tag="bc_sb")
                nc.vector.tensor_copy(bc_sb[:], st['aux'][:])
                nc.vector.tensor_mul(outn[:, st['qcol']:st['qcol'] + 512],
                                     st['ops'][:], bc_sb[:])
                if st['y_st'] is not None:
                    emit_y_tile(st['y_st'], psy, tail=False)

            # av/den emission lags two exp-pairs behind and carries ACROSS
            # chunk boundaries, so the PE always has score work queued
            # between a chunk's last ACT and the next chunk's first one.
            ready = []

            def emit_avden():
                pair, ats, st = ready.pop(0)
                for slot, (src, t, c0) in enumerate(pair):
                    off = 512 * slot
                    w = 512 - c0
                    vt_l = (kbv_sb if src == 'kb' else
                            vkm)[:, 128 * t:128 * t + 128]
                    nav, nst = st['nav'], st['nst']
                    nc.tensor.matmul(st['ops'][:, c0:512], vt_l,
                                     ats[:, off:off + w],
                                     start=(nav == 0),
                                     stop=(nav == nst - 1))
                    if src == 'kb':
                        # KB denominator: PE accumulation group
                        nc.tensor.matmul(st['aux'][0:1, 0:512],
                                         onb_sb[:, 0:1],
                                         ats[:, off:off + 512],
                                         start=(nav == 0), stop=False)
                    else:
                        st['deferred'].append((ats, off, w, c0, nav == 8))
                    st['nav'] += 1

            def attn_chunk(c, i, pssc, psy):
                qcol = 1024 * i + 512 * c
                steps = [('kb', t, 0) for t in range(8)] + \
                        [('sf', t, col0_map[(t, c)])
                         for t in self_tiles[c]]
                nst = len(steps)
                assert nst % 2 == 0
                npair = nst // 2
                st = dict(ops=psout.tile([128, 512], f32, tag="out",
                                         name="ops"),
                          aux=psaux.tile([128, 512], f32, tag="aux",
                                         name="aux"),
                          acc=nrm.tile([128, 512], bf16, tag="acc",
                                       name="acc"),
                          qcol=qcol, nav=0, nst=nst, deferred=[], y_st=None)

                for p in range(npair):
                    pair = steps[2 * p:2 * p + 2]
                    ps = pssc.tile([128, 1024], f32, tag="pair")
                    for slot, (src, t, c0) in enumerate(pair):
                        off = 512 * slot
                        w = 512 - c0
                        if src == 'kb':
                            lhsT = kbk_sb[:, 128 * t:128 * t + 128]
                            rhs = qnT[:, qcol:qcol + 512]
                        else:
                            lhsT = kTr[:, 128 * t:128 * t + 128]
                            rhs = qTr[:, qcol + c0:qcol + 512]
                        nc.tensor.matmul(ps[:, off:off + w], lhsT, rhs,
                                         start=True, stop=True)
                        if src == 'sf' and (t, c) in mixed_idx:
                            k = mixed_idx[(t, c)]
                            nc.vector.tensor_add(
                                ps[:, off:off + w], ps[:, off:off + w],
                                mask_sb[:, 512 * k:512 * k + w])
                    # one ACT for the pair, trimmed to the written span
                    w1 = 512 - pair[1][2]
                    ats = atp.tile([128, 1024], bf16, tag="at")
                    nc.scalar.activation(
                        ats[:, 0:512 + w1], ps[:, 0:512 + w1],
                        mybir.ActivationFunctionType.Exp, scale=SCALE)
                    ready.append((pair, ats, st))
                    if p == 2:
                        # previous chunk's finish slots in once this
                        # chunk's pipeline is in flight (its last avden
                        # pairs were emitted at p=0/p=1)
                        flush_finish(psy)
                    if len(ready) > 2:
                        emit_avden()
                pending.append(st)

            # c=1 first with a 3-deep score pipeline (y PSUM not needed
            # yet); its y tiles (4..7) then become PE filler during the
            # ACT-bound c=0 chunks; y(0..3) drain at the end.
            with tc.tile_pool(name="pssc3", bufs=3, space="PSUM") as pssc3:
                for i in range(4):
                    attn_chunk(1, i, pssc3, None)

            with tc.tile_pool(name="pssc2", bufs=2, space="PSUM") as pssc2, \
                 tc.tile_pool(name="psy", bufs=2, space="PSUM") as psy:
                for i in range(4):
                    # ride y tile 4+i (ready once c=1 head 3 is normed)
                    # on the flush that runs inside this chunk
                    pending[-1]['y_st'] = 4 + i
                    attn_chunk(0, i, pssc2, psy)
                while ready:
                    emit_avden()
                # flush the last c=0 chunk, then drain the remaining y
                flush_finish(psy)
                for st in range(0, 4):
                    emit_y_tile(st, psy, tail=True)

    nc.compile()
    return nc


def kernel(hidden_states, attention_mask, position_ids, kb_keys, kb_values,
           Wq, Wq_new, Wk, Wv, Wo):
    import ml_dtypes
    from concourse.bass_utils import run_bass_kernel_spmd

    bf16 = ml_dtypes.bfloat16
    hidden_states = np.asarray(hidden_states, dtype=np.float32)
    attention_mask = np.asarray(attention_mask, dtype=np.float32)
    position_ids = np.asarray(position_ids)
    kb_keys = np.asarray(kb_keys, dtype=np.float32)
    kb_values = np.asarray(kb_values, dtype=np.float32)
    Wq = np.asarray(Wq, dtype=np.float32)
    Wq_new = np.asarray(Wq_new, dtype=np.float32)
    Wk = np.asarray(Wk, dtype=np.float32)
    Wv = np.asarray(Wv, dtype=np.float32)
    Wo = np.asarray(Wo, dtype=np.float32)

    # ---- host: classify self-attention mask blocks ----
    mask = attention_mask[:, 0]  # (B, S, S) [q, key]
    self_tiles = {}
    mixed = []
    col0_map = {}
    for c in range(2):
        tiles = []
        for t in range(8):
            blk = mask[:, 512 * c:512 * c + 512, 128 * t:128 * t + 128]
            if np.all(blk <= -1e8):
                continue
            tiles.append(t)
            # leading q-columns fully masked in every batch can be skipped
            colmask = np.all(blk <= -1e8, axis=(0, 2))  # (512,) per q-col
            col0 = 0
            while col0 < 512 and colmask[col0]:
                col0 += 1
            col0 = (col0 // 128) * 128  # keep 128-aligned for tidy tiles
            col0_map[(t, c)] = col0
            if np.any(blk[:, col0:, :] < 0):
                mixed.append((t, c))
        self_tiles[c] = tiles
    mixed_idx = {tc_: k for k, tc_ in enumerate(mixed)}
    n_mask = len(mixed)

    nc = _build_program(self_tiles, mixed_idx, n_mask, col0_map)

    # ---- host: shared constant prep ----
    inv_freq = 1.0 / (THETA ** (np.arange(0, HD, 2, dtype=np.float32) / HD))
    P = np.zeros((HD, HD), np.float32)
    for d in range(64):
        P[d, d + 64] = -1.0
        P[d + 64, d] = 1.0
    ropePT = np.ascontiguousarray(P.T)
    onesb = np.ones((128, 128), bf16)
    onesf = np.ones((128, 128), np.float32)
    identb = np.eye(128, dtype=np.float32).astype(bf16)

    def pack_w(wT, ndt):
        # wT (H, 128*ndt) -> (128, 2048*ndt): tile (dt) block holds 16
        # h-tiles side by side: cols 2048*dt + 128*h = wT[128h:+128, 128dt:+128]
        out = np.empty((128, 2048 * ndt), bf16)
        for dt_i in range(ndt):
            for h in range(16):
                out[:, 2048 * dt_i + 128 * h:2048 * dt_i + 128 * h + 128] = \
                    wT[128 * h:128 * h + 128, 128 * dt_i:128 * dt_i + 128]
        return out

    cosTs, sinTs, maskTs = [], [], []
    for b in range(B):
        freqs = position_ids[b].astype(np.float32)[:, None] * inv_freq[None, :]
        emb = np.concatenate([freqs, freqs], axis=1)  # (S, 128)
        cosTs.append(np.ascontiguousarray(np.cos(emb).T.astype(np.float32)))
        sinTs.append(np.ascontiguousarray(np.sin(emb).T.astype(np.float32)))
        if n_mask:
            mt = np.zeros((128, 512 * n_mask), bf16)
            for (t, c), k in mixed_idx.items():
                c0 = col0_map[(t, c)]
                w = 512 - c0
                mt[:, 512 * k:512 * k + w] = \
                    mask[b, 512 * c + c0:512 * c + 512,
                         128 * t:128 * t + 128].T
            maskTs.append(mt)

    in_maps = []
    for cid in range(8):
        b, g = cid // 4, cid % 4
        kbv_p = np.empty((128, KB), bf16)
        kvb = kb_values[b, :, 128 * g:128 * g + 128].astype(bf16)
        for t in range(8):
            kbv_p[:, 128 * t:128 * t + 128] = kvb[128 * t:128 * t + 128, :]
        wo_p = np.empty((128, 8192), np.float32)
        woT = Wo[:, 512 * g:512 * g + 512].T  # (512, 2048)
        for i in range(4):
            wo_p[:, 2048 * i:2048 * i + 2048] = woT[128 * i:128 * i + 128, :]
        m = dict(
            xT=np.ascontiguousarray(hidden_states[b].T).astype(bf16),
            wq=pack_w(Wq[512 * g:512 * g + 512, :].T.astype(bf16), 4),
            wqn=pack_w(Wq_new[512 * g:512 * g + 512, :].T.astype(bf16), 4),
            wk=pack_w(Wk[128 * g:128 * g + 128, :].T.astype(bf16), 1),
            wv=pack_w(Wv[128 * g:128 * g + 128, :].T.astype(bf16), 1),
            wo=wo_p,
            kbkT=np.ascontiguousarray(
                kb_keys[b, :, 128 * g:128 * g + 128].T).astype(bf16),
            kbv=kbv_p,
            cosT=cosTs[b], sinT=sinTs[b],
            ropePT=ropePT, onesb=onesb, onesf=onesf, identb=identb,
        )
        if n_mask:
            m['masks'] = maskTs[b]
        in_maps.append(m)

    res = run_bass_kernel_spmd(nc, in_maps, core_ids=list(range(8)))
    if res.exec_time_ns is not None:
        print(f"HW exec time: {res.exec_time_ns} ns")

    out = np.zeros((B, S, H), np.float32)
    for cid in range(8):
        b = cid // 4
        out[b] += res.results[cid]["y"]
    return out
